# revision 9
# baseline (speedup 1.0000x reference)
# Per-sample 256-bin histogram entropy on trn2 (8 cores, data-parallel over batch).
#
# Algorithm (per core, 8 samples of 786432 f32 each):
#   1. DMA f32 tiles into an SBUF arena (whole sample resident).
#   2. Per-sample min/max: DVE free-dim reduce (stride-subsampled) +
#      GPSIMD partition_all_reduce to broadcast across partitions.
#   3. t = (x + (-min)) * (256/range) in [0, 256] (ACT engine, f32->f16);
#      k = floor(t/16) via round-nearest int convert (DVE); v = t - 16k.
#   4. Step matrices in "element-slot" form (col m = 8*i + e, 8 slots):
#      hi[i] = [t >= 16i], lo[j] = [v >= j], i,j = 0..15 (bf16 0/1).
#      i=0/j=0 blocks are all-ones, preset ONCE in persistent double-buffered
#      slabs; i=1..7 + lo j=1..15 on DVE is_ge (4x mode), hi i=8..15 on ACT
#      saturated sigmoid (exact: t-grid is 0.25 there, sigmoid zone ~0.001).
#   5. PE matmuls accumulate O[m,n] = sum_k hi[k,m] lo[k,n] into PSUM:
#      diagonal slots give C[i,j] = #{hi >= i AND lo >= j}.
#   6. Host: 2D difference of C -> 256-bin histogram -> entropy -> mean.
#
# Engine notes (hardware-measured): GPSIMD tensor_scalar on the slab layout
# is ~16us/op AND starves DVE via the shared SBUF port -- never put bulk
# elementwise there. DVE is_ge f16->bf16 hits 4x mode (~327ns per [128,1024]).
# ACT sigmoid ~1.06us. PE pipeline: LDWEIGHTS overlaps MATMUL, ~56-73ns/group.
import numpy as np

P = 128          # SBUF partitions
NB = 16          # bins per level (16 hi x 16 lo = 256)
ES = 8           # element slots per matmul column block
NCORES = 8
BATCH = 64
SPC = BATCH // NCORES          # samples per core
NPS = 3 * 512 * 512            # elements per sample
FPS = NPS // P                 # free-dim length per sample = 6144


def build_nc(spc=SPC, fps=FPS, w=1024, ch=2048, act_hi=8, act_lo=0,
             mm_stride=4, cvt_bias=-0.5 + 2**-16, skip_min=True,
             thr_major=False):
    # act_hi/act_lo: how many of the top hi/lo thresholds run on the ACT
    #   engine (saturated sigmoid). act_hi <= 8 keeps the sigmoid exact
    #   (t >= 128 where the f16 grid is coarser than the sigmoid zone).
    # mm_stride: min/max reduce subsample stride. Narrows the range by
    #   O(stride/N) quantiles; expected <= ~3 elements/sample fall outside
    #   and land in a wrong bin -- far inside the 2e-2 tolerance.
    # skip_min: drop the clamp of k to 15. Only elements with t in
    #   [255.875, 256] (f16-rounded to 256.0) move bin 255 -> 240;
    #   ~4e-4 of a sample, ~1e-4 bits of entropy.
    import concourse.bacc as bacc
    import concourse.mybir as mybir
    import concourse.tile as tile
    from concourse import bass_isa

    assert fps % w == 0 and w % ES == 0 and fps % ch == 0
    g = w // ES                # matmul groups per macro-tile
    nmacro = fps // w
    f32 = mybir.dt.float32
    f16 = mybir.dt.float16
    bf16 = mybir.dt.bfloat16
    i16 = mybir.dt.int16
    Alu = mybir.AluOpType
    Act = mybir.ActivationFunctionType
    X = mybir.AxisListType.X

    nc = bacc.Bacc(None, target_bir_lowering=False, debug=False)
    x_in = nc.declare_dram_parameter("x", [spc, P, fps], f32, isOutput=False)
    c_out = nc.declare_dram_parameter("cmat", [spc, P, P], f32, isOutput=True)

    with tile.TileContext(nc) as tc:
        with (
            tc.tile_pool(name="xf", bufs=2) as x_pool,
            tc.tile_pool(name="tv", bufs=2) as tv_pool,
            tc.tile_pool(name="small", bufs=2) as small_pool,
            tc.tile_pool(name="co", bufs=2) as co_pool,
            tc.tile_pool(name="const", bufs=1) as const_pool,
            tc.tile_pool(name="psum", bufs=2, space="PSUM") as psum_pool,
        ):
            # sigmoid bias tiles: thresholds shifted off the fp16 value grid
            # so the boundary zone never contains a representable t/v
            actb_hi = {}
            actb_lo = {}
            for i in range(NB - act_hi, NB):
                b = const_pool.tile([P, 1], f32, tag=f"abh{i}", name=f"abh{i}")
                nc.vector.memset(b[:], -4096.0 * (16.0 * i - 0.06))
                actb_hi[i] = b
            for j in range(NB - act_lo, NB):
                b = const_pool.tile([P, 1], f32, tag=f"abl{j}", name=f"abl{j}")
                nc.vector.memset(b[:], -4096.0 * (j - 0.03))
                actb_lo[j] = b

            # persistent double-buffered step slabs; block 0 (i=0/j=0) is
            # all-ones and written exactly once here.
            # thr_major: [P, thr, g, slot] -- every step op writes a fully
            # contiguous [P, w] region (candidate for engine accel modes);
            # the matmul then reads a strided [P, (thr, slot)] AP per group.
            # Flattened column order (thr, slot) = 8*i + e either way, so
            # postprocess is layout-agnostic.
            slabs = []
            for b in range(2):
                shape = [P, NB, g, ES] if thr_major else [P, g, P]
                hi_s = const_pool.tile(shape, bf16, tag=f"hs{b}", name=f"hs{b}")
                lo_s = const_pool.tile(shape, bf16, tag=f"ls{b}", name=f"ls{b}")
                # gpsimd memset: idle at kernel head, 54ns vs 911ns on DVE
                if thr_major:
                    nc.gpsimd.memset(hi_s[:, 0], 1.0)
                    nc.gpsimd.memset(lo_s[:, 0], 1.0)
                else:
                    nc.gpsimd.memset(hi_s[:, :, 0:ES], 1.0)
                    nc.gpsimd.memset(lo_s[:, :, 0:ES], 1.0)
                slabs.append((hi_s, lo_s))

            for s in range(spc):
                # ---- phase A: load + min/max + scale factors ----
                xt = x_pool.tile([P, fps], f32, tag="xt")
                for c in range(0, fps, ch):
                    nc.sync.dma_start(out=xt[:, c : c + ch], in_=x_in[s, :, c : c + ch])
                mx = small_pool.tile([P, 1], f32, tag="mx")
                mn = small_pool.tile([P, 1], f32, tag="mn")
                xsub = xt[:, ::mm_stride] if mm_stride > 1 else xt[:]
                nc.vector.tensor_reduce(mx[:], xsub, axis=X, op=Alu.max)
                nc.vector.tensor_reduce(mn[:], xsub, axis=X, op=Alu.min)
                nmn = small_pool.tile([P, 1], f32, tag="nmn")
                nc.vector.tensor_scalar_mul(nmn[:], mn[:], -1.0)
                # cross-partition: all partitions end up with the global value
                mxr = small_pool.tile([P, 1], f32, tag="mxr")
                nmnr = small_pool.tile([P, 1], f32, tag="nmnr")
                nc.gpsimd.partition_all_reduce(
                    mxr[:], mx[:], channels=P, reduce_op=bass_isa.ReduceOp.max
                )
                nc.gpsimd.partition_all_reduce(
                    nmnr[:], nmn[:], channels=P, reduce_op=bass_isa.ReduceOp.max
                )
                rng = small_pool.tile([P, 1], f32, tag="rng")
                nc.vector.tensor_tensor(rng[:], mxr[:], nmnr[:], op=Alu.add)
                rcp = small_pool.tile([P, 1], f32, tag="rcp")
                nc.vector.reciprocal(rcp[:], rng[:])
                sc = small_pool.tile([P, 1], f32, tag="sc")
                nc.vector.tensor_scalar_mul(sc[:], rcp[:], 256.0)
                nmnsc = small_pool.tile([P, 1], f32, tag="nmnsc")
                nc.vector.tensor_tensor(nmnsc[:], nmnr[:], sc[:], op=Alu.mult)

                # ---- phase B: binning ----
                cm = psum_pool.tile([P, P], f32, tag="cm")
                for m in range(nmacro):
                    xs = xt[:, m * w : (m + 1) * w]
                    tt = tv_pool.tile([P, w], f16, tag="tt")
                    hi16 = tv_pool.tile([P, w], i16, tag="hi16")
                    vv = tv_pool.tile([P, w], f16, tag="vv")
                    # t = (x + nmn) * sc in [0, 256]; t >= 0 so Abs == identity
                    nc.scalar.activation(tt[:], xs, Act.Abs, bias=nmnsc[:], scale=sc[:])
                    # k = floor(t/16) via round-nearest int convert
                    nc.vector.tensor_scalar(
                        hi16[:], tt[:], 0.0625, cvt_bias, op0=Alu.mult, op1=Alu.add
                    )
                    if not skip_min:
                        nc.vector.tensor_scalar_min(hi16[:], hi16[:], 15)
                    # v = t - 16*floor(t/16) in [0, 16]
                    nc.vector.scalar_tensor_tensor(
                        out=vv[:], in0=hi16[:], scalar=-16.0, in1=tt[:],
                        op0=Alu.mult, op1=Alu.add,
                    )
                    hi_slab, lo_slab = slabs[(s * nmacro + m) % 2]
                    if thr_major:
                        t3, v3 = tt[:], vv[:]
                        hi_dsts = [
                            hi_slab[:, i].rearrange("p g e -> p (g e)")
                            for i in range(NB)
                        ]
                        lo_dsts = [
                            lo_slab[:, i].rearrange("p g e -> p (g e)")
                            for i in range(NB)
                        ]
                    else:
                        t3 = tt[:].rearrange("p (g e) -> p g e", e=ES)
                        v3 = vv[:].rearrange("p (g e) -> p g e", e=ES)
                        hi_dsts = [
                            hi_slab[:, :, ES * i : ES * (i + 1)] for i in range(NB)
                        ]
                        lo_dsts = [
                            lo_slab[:, :, ES * i : ES * (i + 1)] for i in range(NB)
                        ]
                    for i in range(1, NB):
                        # saturated sigmoid: sigmoid(4096*(t-thr)) is exactly
                        # 0.0/1.0 in bf16 outside a ~0.001-wide boundary zone
                        if i >= NB - act_hi:
                            nc.scalar.activation(
                                hi_dsts[i], t3, Act.Sigmoid,
                                bias=actb_hi[i][:], scale=4096.0,
                            )
                        else:
                            nc.vector.tensor_scalar(
                                hi_dsts[i], t3, 16.0 * i, None, op0=Alu.is_ge
                            )
                        if i >= NB - act_lo:
                            nc.scalar.activation(
                                lo_dsts[i], v3, Act.Sigmoid,
                                bias=actb_lo[i][:], scale=4096.0,
                            )
                        else:
                            nc.vector.tensor_scalar(
                                lo_dsts[i], v3, float(i), None, op0=Alu.is_ge
                            )
                    for gi in range(g):
                        if thr_major:
                            lhsT = hi_slab[:, :, gi, :]
                            rhs = lo_slab[:, :, gi, :]
                        else:
                            lhsT = hi_slab[:, gi, :]
                            rhs = lo_slab[:, gi, :]
                        nc.tensor.matmul(
                            cm[:],
                            lhsT,
                            rhs,
                            start=(m == 0 and gi == 0),
                            stop=(m == nmacro - 1 and gi == g - 1),
                        )
                co = co_pool.tile([P, P], f32, tag="co")
                nc.scalar.activation(co[:], cm[:], Act.Copy)
                nc.sync.dma_start(out=c_out[s], in_=co[:])
    nc.compile()
    return nc


def postprocess(cmats, n_per_sample):
    """cmats: [nsamples, P, P] f32 matmul outputs -> list of entropies (bits)."""
    ents = []
    for O in cmats:
        O4 = O.reshape(NB, ES, NB, ES)
        C2 = np.einsum("iaja->ij", O4)  # sum diagonal element slots
        Cp = np.zeros((NB + 1, NB + 1))
        Cp[:NB, :NB] = C2
        h = Cp[:NB, :NB] - Cp[1:, :NB] - Cp[:NB, 1:] + Cp[1:, 1:]
        hist = h.reshape(NB * NB)
        total = hist.sum()
        p = hist / total
        nz = p > 0
        ents.append(-(p[nz] * np.log2(p[nz])).sum())
    return ents


_NC_CACHE = {}

BEST_CFG = dict(act_hi=8, act_lo=0, mm_stride=4, skip_min=True)

SWEEP_CFGS = [
    ("mm16", dict(act_hi=8, act_lo=0, mm_stride=16)),
    ("mm64", dict(act_hi=8, act_lo=0, mm_stride=64)),
    ("mm64a7", dict(act_hi=7, act_lo=0, mm_stride=64)),
    ("mm64a6", dict(act_hi=6, act_lo=0, mm_stride=64)),
]


def kernel(y_pred: np.ndarray) -> np.ndarray:
    from concourse.bass_utils import run_bass_kernel_spmd

    assert y_pred.shape == (BATCH, 3, 512, 512) and y_pred.dtype == np.float32
    x = np.ascontiguousarray(y_pred).reshape(NCORES, SPC, P, FPS)
    in_maps = [{"x": x[c]} for c in range(NCORES)]
    if "nc" not in _NC_CACHE:
        _NC_CACHE["nc"] = build_nc(**BEST_CFG)
    res = run_bass_kernel_spmd(_NC_CACHE["nc"], in_maps, list(range(NCORES))).results
    ents = []
    for c in range(NCORES):
        ents.extend(postprocess(res[c]["cmat"], NPS))
    return np.array(np.mean(ents), dtype=np.float32)


if __name__ == "__main__":
    import reference

    inputs = reference.setup_inputs()
    y = np.asarray(inputs["y_pred"])
    out = kernel(y)
    print("kernel out:", out)
